# revision 12
# baseline (speedup 1.0000x reference)
"""Segment-mean pooling (CSR pointer) on 8 Trainium2 NeuronCores.

Strategy (data-parallel over nodes, per sharding hint):
  - Rows of x [N, 128] are split equally across 8 cores (65536 rows each).
  - Host precomputes, per 128-row tile, a one-hot "piece" matrix mapping each
    row to the (<= 8) distinct segments intersecting that tile.
  - Each core streams its x shard through the PE: per tile,
    piece_sums[feat, piece] = x_tile.T @ onehot_tile  (fp32 matmul into PSUM).
  - Host scatter-adds the tiny per-tile piece sums into the [1024, 128]
    segment sums (the "all-reduce over partials"), then divides by counts.
"""

import os
import numpy as np

P = 128            # rows per tile == SBUF partitions
PIECES = 8         # max distinct segments per tile handled on device
CHUNK_T = 16       # tiles per x DMA (16 * 128 * 128 * 4B = 1 MB)
TILES_PER_BANK = 64  # 64 tiles * 8 pieces * 4B = 2 KB/partition = 1 PSUM bank
XBUFS = 8          # x chunk ring = one full DMAHW lane rotation
N_CORES = 8

_CACHE = {}
LAST_RESULTS = None  # BassKernelResults of the most recent device run


def _prune_implied_waits(nc):
    """Walrus on this compile path allows at most ONE sync wait per
    engine instruction (it has no wait-splitting pass). The Tile layer
    emits semantically-redundant waits: an x-chunk DMA reusing a buffer
    waits both on PE (WAR vs. the matmuls that read the old contents)
    and on the old chunk DMA's completion sem (WAW + lane recycle) —
    but the matmuls themselves waited on that DMA sem, so the PE wait
    transitively implies it.

    Sound pruning rule (pure semaphore arithmetic, order-independent):
    a wait (S >= v) on instruction D is implied by D's wait (A >= va)
    if some instruction whose cumulative post-update value of sem A is
    <= va carries an explicit wait (S >= v') with v' >= v. Sem updates
    post at instruction completion, so A >= va proves that instruction
    completed, hence (S >= v') held, hence (S >= v).
    """
    GE = "sem-ge-imm"
    all_insts = []
    for f in nc.m.functions:
        for blk in f.blocks:
            all_insts.extend(blk.instructions)

    cum = {}
    records = []  # (post_sem, post_value, [(wait_sem, wait_value), ...])
    for i in all_insts:
        si = getattr(i, "sync_info", None)
        if si is None:
            continue
        waits = [
            (w.ant_name, w.wait_value)
            for w in (si.on_wait or [])
            if w.wait_mode == GE
        ]
        for u in si.on_update or []:
            if u.update_mode in ("sem-inc", "sem-add-imm"):
                cum[u.ant_name] = cum.get(u.ant_name, 0) + u.update_value
                if waits:
                    records.append((u.ant_name, cum[u.ant_name], waits))

    def implied(anchor_sem, anchor_val, s, v):
        for ps_, pv, ws in records:
            if ps_ == anchor_sem and pv <= anchor_val:
                for s2, v2 in ws:
                    if s2 == s and v2 >= v:
                        return True
        return False

    leftover = []
    for i in all_insts:
        tname = type(i).__name__
        if tname in ("InstDrain", "InstEventSemaphore"):
            continue  # drains are lowered specially; event sems allow 2
        si = getattr(i, "sync_info", None)
        if si is None or not si.on_wait or len(si.on_wait) <= 1:
            continue
        # dedup identical (sem, value) pairs (WAW and lane-recycle collide)
        uniq = {}
        for w in si.on_wait:
            key = (w.ant_name, w.wait_mode, w.wait_value)
            uniq.setdefault(key, w)
        waits = list(uniq.values())
        if len(waits) > 1:
            anchors = sorted(
                waits, key=lambda w: (not w.ant_name.startswith("PE"), w.ant_name)
            )
            for a in anchors:
                if a.wait_mode != GE:
                    continue
                rest = [w for w in waits if w is not a]
                if all(
                    w.wait_mode == GE
                    and implied(a.ant_name, a.wait_value, w.ant_name, w.wait_value)
                    for w in rest
                ):
                    waits = [a]
                    break
        si.on_wait = waits
        if len(waits) > 1:
            leftover.append((tname, getattr(i, "name", "?"), waits))
    if leftover:
        detail = "; ".join(f"{t} {n}: {len(w)} waits" for t, n, w in leftover[:8])
        raise RuntimeError(f"unprunable multi-wait instructions: {detail}")


def _split_drain_waits(nc):
    """Walrus also rejects >1 wait on InstDrain. A drain's waits are a
    pure AND; instructions on one engine queue execute in order, so an
    N-wait drain == N consecutive single-wait drains on that engine."""
    import copy

    for f in nc.m.functions:
        for blk in f.blocks:
            new = []
            for i in blk.instructions:
                si = getattr(i, "sync_info", None)
                if (
                    type(i).__name__ == "InstDrain"
                    and si is not None
                    and si.on_wait
                    and len(si.on_wait) > 1
                ):
                    waits = list(si.on_wait)
                    for k, w in enumerate(waits):
                        c = copy.deepcopy(i)
                        c.sync_info.on_wait = [w]
                        if k < len(waits) - 1:
                            c.sync_info.on_update = []
                        c.name = f"{i.name}s{k}"
                        new.append(c)
                else:
                    new.append(i)
            blk.instructions[:] = new


def _build_program(T, t_process=None):
    """One Bass program, identical on all cores. T = tiles per core.

    t_process < T processes only a prefix (for differential wall-clock
    timing with identical H2D cost); the graded path uses t_process=T.

    Wait-legality plan (1 wait per instruction, see _prune_implied_waits):
      - x chunks cycle a ring of XBUFS=8 buffers == one full DMAHW lane
        rotation, and output DMAs go via gpsimd (SWDGE lanes), so a chunk
        DMA's WAW and lane-recycle deps collapse onto the SAME (sem,
        value) — its previous-ring-slot DMA — which the pruning pass
        removes as implied by the PE WAR wait.
      - one guard matmul absorbs the onehot-DMA wait, so real matmuls
        carry only their x-chunk RAW wait (PE covered-clock does the rest).
      - psum/outs pools have >= n_banks buffers: no slot reuse, so the
        DVE copy waits only on PE, the out DMA only on DVE.
    """
    import concourse.tile as tile
    from concourse import bass, mybir

    if t_process is None:
        t_process = T
    assert t_process % TILES_PER_BANK == 0 and TILES_PER_BANK % CHUNK_T == 0
    n_banks = t_process // TILES_PER_BANK
    assert n_banks <= 8

    nc = bass.Bass()
    x_dram = nc.declare_dram_parameter("x", [T * P, P], mybir.dt.float32, isOutput=False)
    oh_dram = nc.declare_dram_parameter(
        "onehot", [P, T, PIECES], mybir.dt.float32, isOutput=False
    )
    out_dram = nc.declare_dram_parameter(
        "out", [P, T, PIECES], mybir.dt.float32, isOutput=True
    )

    # [T*P, 128] row-major -> [p, t, f] view: partition = row-within-tile
    xr = x_dram.rearrange("(t p) f -> p t f", p=P)

    with tile.TileContext(nc) as tc:
        with (
            tc.tile_pool(name="xin", bufs=XBUFS) as xpool,
            tc.tile_pool(name="oh", bufs=1) as ohpool,
            tc.tile_pool(name="outs", bufs=8) as opool,
            tc.tile_pool(name="psum", bufs=8, space="PSUM") as psum,
        ):
            oh_sb = ohpool.tile([P, T, PIECES], mybir.dt.float32)
            nc.sync.dma_start(oh_sb[:], oh_dram[:])

            chunks_per_bank = TILES_PER_BANK // CHUNK_T
            for b in range(n_banks):
                ps = psum.tile(
                    [P, TILES_PER_BANK, PIECES], mybir.dt.float32, name="ps"
                )
                if b == 0:
                    # guard matmul: takes the onehot-DMA wait; its output
                    # region is overwritten by the first real matmul.
                    nc.tensor.matmul(
                        out=ps[0:PIECES, 0, :],
                        lhsT=oh_sb[:, 0, :],
                        rhs=oh_sb[:, 0, :],
                        start=True,
                        stop=True,
                    )
                for cc in range(chunks_per_bank):
                    t0 = b * TILES_PER_BANK + cc * CHUNK_T
                    xt = xpool.tile([P, CHUNK_T, P], mybir.dt.float32, name="xt")
                    nc.sync.dma_start(xt[:], xr[:, t0 : t0 + CHUNK_T, :])
                    for j in range(CHUNK_T):
                        t = t0 + j
                        nc.tensor.matmul(
                            out=ps[:, t - b * TILES_PER_BANK, :],
                            lhsT=xt[:, j, :],
                            rhs=oh_sb[:, t, :],
                            start=True,
                            stop=True,
                        )
                ob = opool.tile([P, TILES_PER_BANK, PIECES], mybir.dt.float32, name="ob")
                nc.vector.tensor_copy(ob[:], ps[:])
                nc.gpsimd.dma_start(
                    out_dram[:, b * TILES_PER_BANK : (b + 1) * TILES_PER_BANK, :],
                    ob[:],
                )

    nc.finalize()
    _prune_implied_waits(nc)
    _split_drain_waits(nc)
    return nc


def _host_prep(x: np.ndarray, ptr: np.ndarray):
    """Per-tile piece assignment: onehot matrices + piece->segment map."""
    N, D = x.shape
    rows_per_core = N // N_CORES
    T = rows_per_core // P          # tiles per core
    NT = N_CORES * T                # total tiles

    # batch[j] = segment of row j (same formula as the reference)
    batch = np.searchsorted(ptr, np.arange(N, dtype=np.int64), side="right") - 1
    batch_t = batch.reshape(NT, P)

    # dense rank of each row's segment within its tile (batch is sorted)
    newseg = np.zeros((NT, P), dtype=np.int64)
    newseg[:, 1:] = batch_t[:, 1:] != batch_t[:, :-1]
    rank = np.cumsum(newseg, axis=1)          # [NT, P], 0..m-1
    n_pieces = rank[:, -1] + 1
    ok = n_pieces <= PIECES                    # tiles the device handles

    # seg_map[g, k] = global segment id of piece k in tile g (-1 = unused)
    seg_map = np.full((NT, PIECES), -1, dtype=np.int64)
    tflat = np.repeat(np.arange(NT), P)
    okflat = np.repeat(ok, P)
    seg_map[tflat[okflat], rank.ravel()[okflat]] = batch_t.ravel()[okflat]

    # onehot[c, r, t_local, k] = 1 iff row r of tile t has rank k
    onehot = np.zeros((N_CORES, P, T, PIECES), dtype=np.float32)
    c_idx = tflat // T
    tl_idx = tflat % T
    r_idx = np.tile(np.arange(P), NT)
    onehot[c_idx[okflat], r_idx[okflat], tl_idx[okflat], rank.ravel()[okflat]] = 1.0
    return T, batch_t, ok, seg_map, onehot


def kernel(x: np.ndarray, pointer: np.ndarray) -> np.ndarray:
    global LAST_RESULTS
    from concourse.bass_utils import run_bass_kernel_spmd

    x = np.ascontiguousarray(np.asarray(x, dtype=np.float32))
    ptr = np.asarray(pointer).astype(np.int64)
    N, D = x.shape
    B = ptr.shape[0] - 1
    assert D == P and N % (N_CORES * P) == 0
    rows_per_core = N // N_CORES

    T, batch_t, ok, seg_map, onehot = _host_prep(x, ptr)

    key = (T,)
    if key not in _CACHE:
        _CACHE[key] = _build_program(T)
    nc = _CACHE[key]

    in_maps = [
        {
            "x": x[c * rows_per_core : (c + 1) * rows_per_core],
            "onehot": onehot[c],
        }
        for c in range(N_CORES)
    ]
    trace = os.environ.get("POOL_KERNEL_TRACE", "0") == "1"
    res = run_bass_kernel_spmd(nc, in_maps, list(range(N_CORES)), trace=trace)
    LAST_RESULTS = res

    seg_sum = np.zeros((B, D), dtype=np.float64)
    for c in range(N_CORES):
        piece = np.asarray(res.results[c]["out"], dtype=np.float64)  # [P(feat), T, 8]
        vals = piece.transpose(1, 2, 0).reshape(T * PIECES, D)       # [(t,k), feat]
        ids = seg_map[c * T : (c + 1) * T].ravel()
        keep = ids >= 0
        np.add.at(seg_sum, ids[keep], vals[keep])

    # host fallback for (vanishingly rare) tiles with > PIECES segments
    for g in np.nonzero(~ok)[0]:
        rows = slice(g * P, (g + 1) * P)
        np.add.at(seg_sum, batch_t[g], x[rows].astype(np.float64))

    counts = (ptr[1:] - ptr[:-1]).astype(np.float64)
    out = seg_sum / np.maximum(counts, 1.0)[:, None]
    return out.astype(np.float32)
